# revision 14
# baseline (speedup 1.0000x reference)
"""Causal self-attention (B=2, T=2048, C=1024, NH=16, D=64) on 8 TRN2 NeuronCores.

Sharding: 2-way batch x 4-way head-group tensor parallel (4 heads/core).

v2 design (vs v1 baseline at ~208us):
- Scores for a head PAIR are computed with PE row-tiling: head-even's kT
  occupies array rows 0-63 (tile_position (0,0)) and head-odd's rows 64-127
  ((64,0)), so both heads' score matmuls run concurrently -- halving the
  score cost that a K=64 contraction would otherwise waste.
- One scalar-engine exp covers both heads per (k-tile, q-chunk) unit
  ([128, <=1024] activations); the softmax denominator rides as a 65th
  "ones" column of v through the av matmul (no max subtraction needed,
  |scores| <= ~4).
- Unit emission is lag-pipelined (scores lead av by 2 units = the S-tile
  rotation depth) so the PE queue never blocks on the exp of the current
  unit while later scores could run.
- c_proj is split into two host-summed partials (heads 0-1 -> out0,
  heads 2-3 -> out1). out0's matmuls become PE filler during pair-B
  attention; out1's early t-chunks become filler during pair-B's second
  pass. qk projection for pair B is filler during pair-A attention.
- Normalization happens per q-chunk as soon as that chunk's k-loop ends
  (chunks finish at kt=4qc+3), shrinking the pass-boundary PSUM bubble.
- Engine placement: Scalar = exp only; Vector = masks/norm/half the
  PSUM->SBUF casts; GpSimd = qk-bias adds, v copies, other casts;
  Sync = DMA issue.

All matmuls bf16 with fp32 PSUM accumulation.
"""

import numpy as np
import ml_dtypes

import concourse.bass as bass
import concourse.mybir as mybir
import concourse.tile as tile
from concourse import bacc
from concourse.bass_utils import run_bass_kernel_spmd

BF16 = mybir.dt.bfloat16
F32 = mybir.dt.float32

B, T, C = 2, 2048, 1024
NH, D = 16, 64
HPC = NH // 4          # heads per core = 4
CL = HPC * D           # local channels = 256
N_CORES = 8

AF = mybir.ActivationFunctionType

NKT = C // 128        # 8 k-tiles over the C contraction
NTT = T // 128        # 16 t-tiles
NTC = T // 512        # 4 t-chunks


def build_graph():
    nc = bacc.Bacc("TRN2")

    xT_d = nc.declare_dram_parameter("xT", [C, T], BF16, isOutput=False)
    wqk_d = nc.declare_dram_parameter("wqkT", [C, 2 * CL], BF16, isOutput=False)
    wv_d = nc.declare_dram_parameter("wvT", [C, CL], BF16, isOutput=False)
    wp_d = nc.declare_dram_parameter("wpT", [CL, C], BF16, isOutput=False)
    bqk_d = nc.declare_dram_parameter("bqk", [128, 4], F32, isOutput=False)
    bv_d = nc.declare_dram_parameter("bv", [1, CL], BF16, isOutput=False)
    mask_d = nc.declare_dram_parameter("mask", [128, 128], BF16, isOutput=False)
    out0_d = nc.declare_dram_parameter("out0", [C, T], BF16, isOutput=True)
    out1_d = nc.declare_dram_parameter("out1", [C, T], BF16, isOutput=True)
    out_ds = [out0_d, out1_d]

    with tile.TileContext(nc) as tc:
        with (
            tc.tile_pool(name="persist", bufs=1) as pp,
            tc.tile_pool(name="work", bufs=6) as wp,
            tc.tile_pool(name="bcast", bufs=3) as bcp,
            tc.tile_pool(name="dram", bufs=4, space="DRAM") as dpool,
            tc.tile_pool(name="ps", bufs=2, space="PSUM") as ps,
            tc.tile_pool(name="psav", bufs=1, space="PSUM") as psav,
        ):
            # ---- persistent SBUF tiles ----
            xT_sb = [pp.tile([128, T], BF16, tag=f"xT{i}", name=f"xT{i}") for i in range(NKT)]
            wv_sb = [pp.tile([128, CL], BF16, tag=f"wv{i}", name=f"wv{i}") for i in range(NKT)]
            wqk_sb = [pp.tile([128, 2 * CL], BF16, tag=f"wqk{i}", name=f"wqk{i}") for i in range(NKT)]
            wp_sb = [pp.tile([128, C], BF16, tag=f"wp{i}", name=f"wp{i}") for i in range(CL // 128)]
            bqk_sb = pp.tile([128, 4], F32, tag="bqk")
            bv_sb = pp.tile([1, CL], BF16, tag="bv")
            mask_sb = pp.tile([128, 128], BF16, tag="mask")
            ones_sb = pp.tile([1, 128], BF16, tag="ones")
            qkT_sb = [pp.tile([128, T], BF16, tag=f"qk{i}", name=f"qk{i}") for i in range(4)]
            v_sb = [pp.tile([128, HPC * (D + 1)], BF16, tag=f"v{i}", name=f"v{i}") for i in range(NTT)]
            yT_sb = [pp.tile([128, T], BF16, tag=f"y{i}", name=f"y{i}") for i in range(2)]

            # ---- input DMA: weights first, then xT in t-chunk-major order ----
            # spread DMA *issue* cost (~600ns each) across engine queues:
            # sync: xT t0/t2; gpsimd: wv, xT t1/t3, wp; vector: wqk + biases
            for i in range(NKT):
                nc.gpsimd.dma_start(wv_sb[i][:], wv_d[128 * i : 128 * (i + 1), :])
            nc.scalar.dma_start(bv_sb[:], bv_d[:])
            nc.scalar.dma_start(bqk_sb[:], bqk_d[:])
            nc.scalar.dma_start(mask_sb[:], mask_d[:])
            nc.vector.memset(ones_sb[:], 1.0)
            # warm up the scalar engine's exp table-set load (~2.7us) so it
            # overlaps the input DMA instead of delaying the first real exp
            warm_sb = pp.tile([1, 8], F32, tag="warm")
            nc.vector.memset(warm_sb[:], 0.0)
            nc.scalar.activation(warm_sb[:], warm_sb[:], AF.Exp, scale=0.125)
            # PE warmup: ~24 matmuls on memset data release the HAM clock gate
            # (4/8 -> 8/8) during the DMA-bound start, so v-proj runs at 2.4GHz
            wu_sb = pp.tile([128, 512], BF16, tag="wu")
            nc.vector.memset(wu_sb[:], 0.0)
            for r in range(2):
                pw = ps.tile([128, 1024], F32, tag="S", name=f"pw{r}")[:, 0:512]
                for k in range(12):
                    nc.tensor.matmul(
                        pw[:], wu_sb[:, 0:128], wu_sb[:],
                        start=(k == 0), stop=(k == 11),
                    )
            for tcn in range(NTC):
                sl = slice(512 * tcn, 512 * (tcn + 1))
                eng = nc.sync if tcn % 2 == 0 else nc.gpsimd
                for i in range(NKT):
                    eng.dma_start(
                        xT_sb[i][:, sl], xT_d[128 * i : 128 * (i + 1), sl]
                    )
                if tcn == 0:
                    for i in range(NKT):
                        nc.scalar.dma_start(
                            wqk_sb[i][:], wqk_d[128 * i : 128 * (i + 1), :]
                        )
            for i in range(CL // 128):
                nc.gpsimd.dma_start(wp_sb[i][:], wp_d[128 * i : 128 * (i + 1), :])

            # ---- v projection, t-major: psum[t128, 4h*64d] = xT_tile^T @ wvT ----
            def emit_v(tt):
                pv = ps.tile([128, 1024], F32, tag="S", name=f"pv{tt}")[:, 0:CL]
                for kt in range(NKT):
                    nc.tensor.matmul(
                        pv[:],
                        xT_sb[kt][:, 128 * tt : 128 * (tt + 1)],
                        wv_sb[kt][:],
                        start=(kt == 0),
                        stop=False,
                    )
                nc.tensor.matmul(pv[:], ones_sb[:], bv_sb[:], start=False, stop=True)
                vt = v_sb[tt][:].rearrange("p (h d) -> p h d", h=HPC)
                src_ap = pv[:].rearrange("p (h d) -> p h d", h=HPC)
                if tt % 2 == 0:
                    nc.scalar.copy(vt[:, :, 0:D], src_ap)
                else:
                    nc.vector.tensor_copy(vt[:, :, 0:D], src_ap)
                nc.gpsimd.memset(vt[:, :, D : D + 1], 1.0)

            # ---- q/k projection, feature-major: psum[f128, t512] ----
            def emit_qk(ft, tcn, ntcn=1):
                pq = ps.tile([128, 1024], F32, tag="S", name=f"pq{ft}{tcn}")[
                    :, 0 : 512 * ntcn
                ]
                for t in range(tcn, tcn + ntcn):
                    for kt in range(NKT):
                        nc.tensor.matmul(
                            pq[:, 512 * (t - tcn) : 512 * (t - tcn + 1)],
                            wqk_sb[kt][:, 128 * ft : 128 * (ft + 1)],
                            xT_sb[kt][:, 512 * t : 512 * (t + 1)],
                            start=(kt == 0),
                            stop=(kt == NKT - 1),
                        )
                nc.vector.tensor_scalar_add(
                    qkT_sb[ft][:, 512 * tcn : 512 * (tcn + ntcn)],
                    pq[:],
                    bqk_sb[:, ft : ft + 1],
                )

            # ---- c_proj partial for head-pair `pair`: out_pair[o,t] = wp_pair^T y_pair
            def emit_cproj(pair, mtp, tcn, dma_eng, copy_eng=None):
                # one group covers out rows [256*mtp, 256*mtp+256)
                po = ps.tile([128, 1024], F32, tag="S", name=f"po{pair}{mtp}{tcn}")
                ysl = yT_sb[pair][:, 512 * tcn : 512 * (tcn + 1)]
                for half in range(2):
                    nc.tensor.matmul(
                        po[:, 512 * half : 512 * (half + 1)],
                        wp_sb[pair][:, 256 * mtp + 128 * half : 256 * mtp + 128 * (half + 1)],
                        ysl,
                        start=True,
                        stop=True,
                    )
                ob = wp.tile([128, 1024], BF16, tag="ob", name=f"ob{pair}{mtp}{tcn}")
                if copy_eng is None:
                    nc.vector.tensor_copy(ob[:], po[:])
                else:
                    copy_eng.copy(ob[:], po[:])
                for half in range(2):
                    mt = 2 * mtp + half
                    dma_eng.dma_start(
                        out_ds[pair][
                            128 * mt : 128 * (mt + 1), 512 * tcn : 512 * (tcn + 1)
                        ],
                        ob[:, 512 * half : 512 * (half + 1)],
                    )

            # ---- attention for one head pair, one pass (q-chunk pair) ----
            # Unit = (kt, qc): S2[128k, 1024] holds h_even scores in [:,0:512],
            # h_odd in [:,512:1024] (row-tiled concurrent matmuls); one exp
            # covers both; av accumulates with the ones-column denominator.
            def attention_pass(pair, qa, qb, filler, fill_every):
                qT = qkT_sb[pair]
                kT = qkT_sb[2 + pair]
                # per-chunk av accumulators [65, 2(heads), 512]; separate tags
                # so the next pass's first chunk never waits on this pass's
                # last-chunk normalization chain
                avs = {
                    qa: psav.tile([D + 1, 2, 512], F32, tag="avA", name=f"avA{pair}{qa}"),
                    qb: psav.tile([D + 1, 2, 512], F32, tag="avB", name=f"avB{pair}{qb}"),
                }
                units = []
                for kt in range(4 * qb + 4):
                    qc0 = kt // 4
                    for qc in (qa, qb):
                        if qc >= qc0:
                            units.append((kt, qc))
                n = len(units)
                LAG = 2
                pend = {}
                fill_ctr = 0

                def emit_scores(i):
                    kt, qc = units[i]
                    so = 128 * kt - 512 * (kt // 4) if qc == kt // 4 else 0
                    S2 = ps.tile([128, 1024], F32, tag="S", name=f"S{pair}{qa}{i}")
                    E2 = wp.tile([128, 1024], BF16, tag="E", name=f"E{pair}{qa}{i}")
                    ksl = slice(128 * kt, 128 * (kt + 1))
                    qsl = slice(512 * qc + so, 512 * (qc + 1))
                    nc.tensor.matmul(
                        S2[:, so:512], kT[0:64, ksl], qT[0:64, qsl],
                        start=True, stop=True,
                    )
                    nc.tensor.matmul(
                        S2[:, 512 + so : 1024], kT[64:128, ksl], qT[64:128, qsl],
                        start=True, stop=True,
                    )
                    # exp of both heads; 3D AP skips the [512:512+so) gap
                    sin = S2[:].rearrange("p (h q) -> p h q", h=2)[:, :, so:512]
                    eout = E2[:].rearrange("p (h q) -> p h q", h=2)[:, :, so:512]
                    nc.scalar.activation(eout, sin, AF.Exp, scale=0.125)
                    if kt // 4 == qc:
                        nc.vector.tensor_mul(
                            E2[:, so : so + 128], E2[:, so : so + 128], mask_sb[:]
                        )
                        nc.vector.tensor_mul(
                            E2[:, 512 + so : 512 + so + 128],
                            E2[:, 512 + so : 512 + so + 128],
                            mask_sb[:],
                        )
                    pend[i] = (E2, kt, qc, so)

                def emit_av(i):
                    E2, kt, qc, so = pend.pop(i)
                    av = avs[qc]
                    for h in range(2):
                        nc.tensor.matmul(
                            av[:, h, so:512],
                            v_sb[kt][:, (D + 1) * (2 * pair + h) : (D + 1) * (2 * pair + h + 1)],
                            E2[:, 512 * h + so : 512 * (h + 1)],
                            start=(kt == 0),
                            stop=(kt == 4 * qc + 3),
                            skip_group_check=True,
                        )
                    if kt == 4 * qc + 3:
                        normalize_chunk(pair, av, qc)

                for i in range(n + LAG):
                    if i < n:
                        emit_scores(i)
                    if i - LAG >= 0:
                        emit_av(i - LAG)
                    fill_ctr += 1
                    if filler and fill_ctr % fill_every == 0:
                        filler.pop(0)()
                while filler:
                    filler.pop(0)()

            # ---- per-chunk normalization: y = av[:D] / av[D] ----
            # Denominators for (h_even, h_odd) are packed to partitions 0/64,
            # reciprocal'd in one DVE op, bounced through DRAM for the
            # partition-broadcast, then two [64,512] multiplies write yT.
            def normalize_chunk(pair, av, qc):
                # free the av PSUM fast: pull numerator + denominators into
                # SBUF on DVE (no DMA deps), then do the DMA-latency-bound
                # broadcast + multiplies on the idle GpSimd queue so the DVE
                # FIFO (causal masks!) never waits on a DMA round trip.
                dn = bcp.tile([65, 512], F32, tag="dn", name=f"dn{pair}{qc}")
                nc.vector.tensor_copy(dn[0:1, :], av[D : D + 1, 0, :])
                nc.vector.tensor_copy(dn[64:65, :], av[D : D + 1, 1, :])
                avc = bcp.tile([64, 1024], BF16, tag="avc", name=f"avc{pair}{qc}")
                nc.vector.tensor_copy(
                    avc[:].rearrange("p (h q) -> p h q", h=2), av[0:D, :, :]
                )
                rc = bcp.tile([65, 512], F32, tag="rc", name=f"rc{pair}{qc}")
                nc.vector.reciprocal_approx_fast(out=rc[:], in_=dn[:])
                scr = dpool.tile([2, 512], F32, tag="scr", name=f"scr{pair}{qc}")
                nc.gpsimd.dma_start(scr[0:1, :], rc[0:1, :])
                nc.gpsimd.dma_start(scr[1:2, :], rc[64:65, :])
                bc = bcp.tile([64, 1024], F32, tag="bc", name=f"bc{pair}{qc}")
                bc_src = bass.AP(
                    tensor=scr[:].tensor,
                    offset=scr[:].offset,
                    ap=[[0, 64], [1, 1024]],
                )
                nc.gpsimd.dma_start(bc[:], bc_src)
                qsl = slice(512 * qc, 512 * (qc + 1))
                for h in range(2):
                    nc.gpsimd.tensor_mul(
                        yT_sb[pair][64 * h : 64 * h + 64, qsl],
                        avc[:, 512 * h : 512 * (h + 1)],
                        bc[:, 512 * h : 512 * (h + 1)],
                    )

            # ================= schedule =================
            for tt in range(12):
                emit_v(tt)
            for ft in (0, 2):
                emit_qk(ft, 0, ntcn=2)

            # pair A (heads 0,1)
            # deadlines: qT chunks 2+3 (qk 0,2 / 0,3) must finish before A-p23
            # starts; kT tiles 8-15 (qk 2,2 / 2,3) by A-p23 units 16/24.
            fillA1 = [lambda f=ft, t=tcn: emit_qk(f, t)
                      for ft, tcn in ((0, 2), (0, 3))]
            fillA1 += [lambda t=tt: emit_v(t) for tt in range(12, 16)]
            attention_pass(0, 0, 1, fillA1, 2)
            fillA2 = [lambda f=ft, t=tcn: emit_qk(f, t)
                      for ft, tcn in ((2, 2), (2, 3), (1, 0), (3, 0), (1, 1), (3, 1))]
            fillA2 += [lambda m=mtp: emit_cproj(0, m, 0, nc.sync)
                       for mtp in range(3)]
            attention_pass(0, 2, 3, fillA2, 2)

            # pair B (heads 2,3)
            fillB1 = [lambda f=ft, t=tcn: emit_qk(f, t)
                      for ft, tcn in ((1, 2), (1, 3), (3, 2), (3, 3))]
            attention_pass(1, 0, 1, fillB1, 2)
            fillB2 = [lambda: emit_cproj(0, 3, 0, nc.sync)]
            fillB2 += [lambda m=mtp, t=tcn: emit_cproj(0, m, t, nc.sync)
                       for tcn in (1, 2, 3) for mtp in range(4)]
            fillB2 += [lambda m=mtp: emit_cproj(1, m, 0, nc.sync)
                       for mtp in range(4)]
            fillB2 += [lambda m=mtp: emit_cproj(1, m, 1, nc.sync)
                       for mtp in range(4)]
            attention_pass(1, 2, 3, fillB2, 1)

            # tail: c_proj-B t2 (ready mid-B-p23), then t3; copies split
            # between scalar and vector halves to halve the rotation period
            def tail_cproj(mtp, tcn):
                po = ps.tile([128, 1024], F32, tag="S", name=f"pt{mtp}{tcn}")
                ysl = yT_sb[1][:, 512 * tcn : 512 * (tcn + 1)]
                for half in range(2):
                    nc.tensor.matmul(
                        po[:, 512 * half : 512 * (half + 1)],
                        wp_sb[1][:, 256 * mtp + 128 * half : 256 * mtp + 128 * (half + 1)],
                        ysl,
                        start=True,
                        stop=True,
                    )
                ob = wp.tile([128, 1024], BF16, tag="ob", name=f"obt{mtp}{tcn}")
                nc.scalar.copy(ob[:, 0:512], po[:, 0:512])
                nc.vector.tensor_copy(ob[:, 512:1024], po[:, 512:1024])
                for half in range(2):
                    mt = 2 * mtp + half
                    nc.sync.dma_start(
                        out_ds[1][128 * mt : 128 * (mt + 1), 512 * tcn : 512 * (tcn + 1)],
                        ob[:, 512 * half : 512 * (half + 1)],
                    )

            for tcn in (2, 3):
                for mtp in range(4):
                    tail_cproj(mtp, tcn)
    nc.finalize()
    return nc


_GRAPH_CACHE = {}


def kernel(x, W_attn, b_attn, W_proj, b_proj, bV, **_unused):
    x = np.asarray(x, dtype=np.float32)
    W_attn = np.asarray(W_attn, dtype=np.float32)
    b_attn = np.asarray(b_attn, dtype=np.float32)
    W_proj = np.asarray(W_proj, dtype=np.float32)
    b_proj = np.asarray(b_proj, dtype=np.float32)
    bV = np.asarray(bV, dtype=np.float32)

    bf = ml_dtypes.bfloat16
    xT = [np.ascontiguousarray(x[b].T).astype(bf) for b in range(B)]
    mask = np.triu(np.ones((128, 128), np.float32)).astype(bf)

    in_maps = []
    for core in range(N_CORES):
        b, g = core // 4, core % 4
        rq = slice(CL * g, CL * (g + 1))
        rk = slice(C + CL * g, C + CL * (g + 1))
        rv = slice(2 * C + CL * g, 2 * C + CL * (g + 1))
        wqkT = np.ascontiguousarray(
            np.concatenate([W_attn[rq].T, W_attn[rk].T], axis=1)
        ).astype(bf)
        wvT = np.ascontiguousarray(W_attn[rv].T).astype(bf)
        wpT = np.ascontiguousarray(W_proj[:, CL * g : CL * (g + 1)].T).astype(bf)
        bqk = np.concatenate([b_attn[rq], b_attn[rk]]).reshape(4, 128).T
        bqk = np.ascontiguousarray(bqk).astype(np.float32)
        bv = (bV[HPC * g : HPC * (g + 1)].reshape(1, CL) + b_attn[rv][None]).astype(bf)
        in_maps.append(
            {
                "xT": xT[b],
                "wqkT": wqkT,
                "wvT": wvT,
                "wpT": wpT,
                "bqk": bqk,
                "bv": bv,
                "mask": mask,
            }
        )

    if "nc" not in _GRAPH_CACHE:
        _GRAPH_CACHE["nc"] = build_graph()
    nc = _GRAPH_CACHE["nc"]
    _GRAPH_CACHE["in_maps"] = in_maps

    res = run_bass_kernel_spmd(nc, in_maps, core_ids=list(range(N_CORES)))

    out = np.empty((B, T, C), dtype=np.float32)
    for b in range(B):
        acc = res.results[4 * b]["out0"].astype(np.float32)
        acc += res.results[4 * b]["out1"].astype(np.float32)
        for g in range(1, 4):
            acc += res.results[4 * b + g]["out0"].astype(np.float32)
            acc += res.results[4 * b + g]["out1"].astype(np.float32)
        out[b] = acc.T + b_proj[None, :]
    return out
